# revision 4
# baseline (speedup 1.0000x reference)
"""Trainium2 Bass kernel for nn_AttentionLayer (4x2048x768, d_k=128, d_v=768).

Sharding (sequence-parallel over keys, data-parallel over batch):
8 cores; core c handles batch b=c//2 with KEY half h=c%2. Each core computes
q for ALL 2048 queries but k/v only for its own 1024 keys, then produces the
partial (unnormalized) attention numerator plus the partial softmax row sum:

    out_core[q, 0:768] = sum_{t in own half} exp(s_qt) * v[t, :]
    out_core[q, 768]   = sum_{t in own half} exp(s_qt)

The host adds the two partials of each batch and normalizes
(out = num/rowsum + bv) — an exact reassociation of the softmax.

bk is dropped entirely: s_it = (q̂_i+bq)·(k̂_t+bk) differs from
(q̂_i+bq)·k̂_t only by a per-query constant, which softmax cancels.

All matmul operands are bf16 (converted on the host); PE rate is the same
1 col/cycle as fp32r but all DMA bytes halve. Rel err vs fp32 ref ~2.6e-3.

DMA model (measured): each [128, X] DMA costs ~128 descriptors x ~16ns of
HW-DGE ring time regardless of X, so inputs are host-repacked partition-
major into the FEWEST, fattest-row DMAs, ordered by first PE use:
  scalar ring: wqk (3KB rows) | x-own c4|c5 pair (4KB) | wv whole (9KB)
  sync ring:   x-own c0|c1, c2|c3 pairs (4KB) | x-oth whole (12KB)
q/k consumes chunks in arrival order [0,1,4,5,2,3]. The output DRAM tensor
is partition-major [128, 16, 769] (host re-transposes) and region stores
alternate rings; the last tile's stores split by partition across rings.

PE: q/k-own -> [kT-cast(t); scores0(t); v(t)] interleaved (exp hides under
    v matmuls) -> q-other -> [scores1(t); out0(qc)] interleaved -> out1.
PSUM: "ps" pool 3x[128,1024] f32 (q/k/v/out, 6 banks) +
      "sc" pool 2x[128,512] f32 (score tiles, 2 banks) = all 8 banks.
"""

import sys

sys.path.insert(0, "/opt/trn_rl_repo")

import numpy as np
import ml_dtypes

B, T, DIN, DK, DV = 4, 2048, 768, 128, 768
NCORES = 8
TOWN = 1024  # own keys per core
CH = DIN // 128  # 6 contraction chunks over d_in
TCH = TOWN // 128  # 8 own-key chunks
QCH = T // 128  # 16 query chunks (all queries)
SCALE = 1.0 / float(np.sqrt(DK))

_CACHE = {}


def _build():
    from contextlib import ExitStack

    from concourse import bacc, mybir, tile

    f32 = mybir.dt.float32
    bf16 = mybir.dt.bfloat16

    nc = bacc.Bacc("TRN2", target_bir_lowering=False, debug=False)

    x_own = nc.dram_tensor("x_own", [128, 3, 2 * TOWN], bf16, kind="ExternalInput").ap()
    x_oth = nc.dram_tensor("x_oth", [128, CH * TOWN], bf16, kind="ExternalInput").ap()
    wqk = nc.dram_tensor("wqk", [128, CH, 2 * DK], bf16, kind="ExternalInput").ap()
    wv = nc.dram_tensor("wv", [128, CH, DV], bf16, kind="ExternalInput").ap()
    bq = nc.dram_tensor("bq", [DK, 1], f32, kind="ExternalInput").ap()
    out = nc.dram_tensor("out", [128, QCH, DV + 1], bf16, kind="ExternalOutput").ap()

    with tile.TileContext(nc) as tc, ExitStack() as ctx:
        consts = ctx.enter_context(tc.tile_pool(name="consts", bufs=1))
        persist = ctx.enter_context(tc.tile_pool(name="persist", bufs=1))
        wpool = ctx.enter_context(tc.tile_pool(name="wpool", bufs=1))
        xpool = ctx.enter_context(tc.tile_pool(name="xpool", bufs=1))
        out_pool = ctx.enter_context(tc.tile_pool(name="out_pool", bufs=4))
        ps_pool = ctx.enter_context(tc.tile_pool(name="ps", bufs=3, space="PSUM"))
        sc_pool = ctx.enter_context(tc.tile_pool(name="sc", bufs=2, space="PSUM"))

        bq_sb = consts.tile([DK, 1], f32)
        nc.gpsimd.dma_start(out=bq_sb[:], in_=bq)

        qT_sb = persist.tile([128, T], bf16)  # [dk, q] all queries, q̂+bq
        kT_sb = persist.tile([128, TOWN], bf16)  # [dk, t-own]
        v_sb = persist.tile([128, TCH, DV + 2], bf16)  # [t-part, chunk, dv|1|pad]
        pT_sb = persist.tile([128, TCH, T], bf16)  # [t-part, chunk, q]

        nc.vector.memset(v_sb[:, :, DV : DV + 2], 1.0)

        # x-own pairs: pair p holds chunks 2p|2p+1 side by side (4KB rows)
        xo_sb = xpool.tile([128, 3, 2 * TOWN], bf16)
        xt_sb = xpool.tile([128, CH, TOWN], bf16)
        wqk_sb = wpool.tile([128, CH, 2 * DK], bf16)
        wv_sb = wpool.tile([128, CH, DV], bf16)

        def xo(c):  # own-x chunk c -> [128, TOWN] slice of the pair tile
            return xo_sb[:, c // 2, (c % 2) * TOWN : (c % 2 + 1) * TOWN]

        # Input DMAs ordered by first PE use, few and fat (see module doc)
        nc.scalar.dma_start(out=wqk_sb[:], in_=wqk)
        nc.sync.dma_start(out=xo_sb[:, 0, :], in_=x_own[:, 0, :])
        nc.scalar.dma_start(out=xo_sb[:, 2, :], in_=x_own[:, 2, :])
        nc.sync.dma_start(out=xo_sb[:, 1, :], in_=x_own[:, 1, :])
        nc.scalar.dma_start(out=wv_sb[:], in_=wv)
        nc.sync.dma_start(out=xt_sb[:], in_=x_oth.rearrange("p (c t) -> p c t", c=CH))

        def emit_scores_t(t, qh):
            # scores^T for one own-key chunk, P^T = exp(scale*s) in bf16.
            # 512-col score tiles (1 PSUM bank each) so exps pipeline.
            for n0 in (0, 512):
                ps_s = sc_pool.tile([128, 512], f32, tag="sc")
                nc.tensor.matmul(
                    ps_s[:],
                    kT_sb[:, t * 128 : (t + 1) * 128],
                    qT_sb[:, qh * 1024 + n0 : qh * 1024 + n0 + 512],
                    start=True,
                    stop=True,
                )
                nc.scalar.activation(
                    pT_sb[:, t, qh * 1024 + n0 : qh * 1024 + n0 + 512],
                    ps_s[:],
                    mybir.ActivationFunctionType.Exp,
                    scale=SCALE,
                )

        def emit_v_t(t):
            # v-projection for one own-key chunk
            ps_v = ps_pool.tile([128, 1024], f32, tag="ps")
            for c in range(CH):
                for n0, n1 in ((0, 512), (512, DV)):
                    nc.tensor.matmul(
                        ps_v[:, n0:n1],
                        xo(c)[:, t * 128 : (t + 1) * 128],
                        wv_sb[:, c, n0:n1],
                        start=(c == 0),
                        stop=(c == CH - 1),
                    )
            nc.vector.tensor_copy(v_sb[:, t, 0:DV], ps_v[:, 0:DV])

        def emit_out_qc(qc):
            # partial numerator + rowsum: out[qc] = sum_t P^T[t,qc].T @ [v|1];
            # copy+store each region as soon as its accumulation stops;
            # stores alternate rings (region0 sync, region1 scalar). The
            # final tile's region1 splits by partition across both rings to
            # shorten the drain tail.
            ps_o = ps_pool.tile([128, 1024], f32, tag="ps")
            o_sb = out_pool.tile([128, DV + 1], bf16, tag="o")
            for reg, (n0, n1) in enumerate(((0, 512), (512, DV + 2))):
                for t in range(TCH):
                    nc.tensor.matmul(
                        ps_o[:, n0:n1],
                        pT_sb[:, t, qc * 128 : (qc + 1) * 128],
                        v_sb[:, t, n0:n1],
                        start=(t == 0),
                        stop=(t == TCH - 1),
                    )
                c1 = min(n1, DV + 1)
                nc.vector.tensor_copy(o_sb[:, n0:c1], ps_o[:, n0:c1])
                if reg == 1 and qc == QCH - 1:
                    nc.sync.dma_start(
                        out=out[0:64, qc, n0:c1], in_=o_sb[0:64, n0:c1]
                    )
                    nc.scalar.dma_start(
                        out=out[64:128, qc, n0:c1], in_=o_sb[64:128, n0:c1]
                    )
                else:
                    eng = nc.sync if reg == 0 else nc.scalar
                    eng.dma_start(out=out[:, qc, n0:c1], in_=o_sb[:, n0:c1])

        # q own-half + k own in chunk-ARRIVAL order; q/k share each loaded
        # lhsT for both 512-blocks
        ps_q0 = ps_pool.tile([128, 1024], f32, tag="ps")
        ps_k = ps_pool.tile([128, 1024], f32, tag="ps")
        C_ORDER = [0, 1, 4, 5, 2, 3]
        for i, c in enumerate(C_ORDER):
            st, sp = i == 0, i == CH - 1
            for n0 in (0, 512):
                nc.tensor.matmul(
                    ps_q0[:, n0 : n0 + 512],
                    wqk_sb[:, c, 0:DK],
                    xo(c)[:, n0 : n0 + 512],
                    start=st,
                    stop=sp,
                )
            for n0 in (0, 512):
                nc.tensor.matmul(
                    ps_k[:, n0 : n0 + 512],
                    wqk_sb[:, c, DK : 2 * DK],
                    xo(c)[:, n0 : n0 + 512],
                    start=st,
                    stop=sp,
                )
        # qT = q̂+bq on scalar, split per 512 so scores t=0 unblocks early;
        # kT casts are per-chunk inside the loop below for the same reason
        for n0 in (0, 512):
            nc.scalar.activation(
                qT_sb[:, n0 : n0 + 512],
                ps_q0[:, n0 : n0 + 512],
                mybir.ActivationFunctionType.Identity,
                bias=bq_sb[:],
            )

        # scores for own queries interleaved with v-projection: the scalar
        # exps (~1.4us/chunk) hide under the v matmuls (~2.4us/chunk)
        for t in range(TCH):
            nc.vector.tensor_copy(
                kT_sb[:, t * 128 : (t + 1) * 128], ps_k[:, t * 128 : (t + 1) * 128]
            )
            emit_scores_t(t, 0)
            emit_v_t(t)

        # q other-half
        ps_q1 = ps_pool.tile([128, 1024], f32, tag="ps")
        for c in range(CH):
            for n0 in (0, 512):
                nc.tensor.matmul(
                    ps_q1[:, n0 : n0 + 512],
                    wqk_sb[:, c, 0:DK],
                    xt_sb[:, c, n0 : n0 + 512],
                    start=(c == 0),
                    stop=(c == CH - 1),
                )
        for n0 in (0, 512):
            nc.scalar.activation(
                qT_sb[:, TOWN + n0 : TOWN + n0 + 512],
                ps_q1[:, n0 : n0 + 512],
                mybir.ActivationFunctionType.Identity,
                bias=bq_sb[:],
            )

        # scores for other-half queries interleaved with out0: exps for
        # half 1 complete long before out1 consumes pT, with no PE stall
        for qc in range(8):
            emit_scores_t(qc, 1)
            emit_out_qc(qc)

        for qc in range(8, 16):
            emit_out_qc(qc)

    nc.compile()
    return nc


def _get_nc():
    if "nc" not in _CACHE:
        _CACHE["nc"] = _build()
    return _CACHE["nc"]


def _make_in_maps(x, Wq, bq, Wk, bk, Wv):
    bf16 = ml_dtypes.bfloat16
    wq = np.asarray(Wq, np.float32).astype(bf16).reshape(CH, 128, DK)
    wk = np.asarray(Wk, np.float32).astype(bf16).reshape(CH, 128, DK)
    base = {
        # partition-major packs: contiguous per-partition rows -> few, fat
        # DMA descriptors (wqk 3KB, wv 9KB, x-own 4KB, x-oth 12KB rows)
        "wqk": np.ascontiguousarray(
            np.concatenate([wq, wk], axis=2).transpose(1, 0, 2)
        ),
        "wv": np.ascontiguousarray(
            np.asarray(Wv, np.float32).astype(bf16).reshape(CH, 128, DV).transpose(1, 0, 2)
        ),
        "bq": np.ascontiguousarray(np.asarray(bq, np.float32).reshape(DK, 1)),
    }
    in_maps = []
    for c in range(NCORES):
        b, h = c // 2, c % 2
        xb = x[b]  # [T, DIN]
        rot = np.concatenate([xb[h * TOWN :], xb[: h * TOWN]], axis=0)
        xT = rot.T.astype(bf16).reshape(CH, 128, T).transpose(1, 0, 2)  # [128,c,t]
        own = xT[:, :, 0:TOWN]  # [128, c, 1024]
        m = dict(base)
        m["x_own"] = np.ascontiguousarray(own.reshape(128, 3, 2 * TOWN))
        m["x_oth"] = np.ascontiguousarray(xT[:, :, TOWN:T].reshape(128, CH * TOWN))
        in_maps.append(m)
    return in_maps


def kernel(x, Wq, bq, Wk, bk, Wv, bv):
    from concourse import bass_utils

    x = np.ascontiguousarray(np.asarray(x, dtype=np.float32))
    nc = _get_nc()
    in_maps = _make_in_maps(x, Wq, bq, Wk, bk, Wv)

    res = bass_utils.run_bass_kernel_spmd(nc, in_maps, core_ids=list(range(NCORES)))

    bv = np.asarray(bv, np.float32).reshape(1, DV)
    outp = np.empty((B, T, DV), dtype=np.float32)
    for b in range(B):
        # out is partition-major [128, qc, 769] -> [qc*128+p, 769]
        p0 = res.results[2 * b]["out"].transpose(1, 0, 2).reshape(T, DV + 1)
        p1 = res.results[2 * b + 1]["out"].transpose(1, 0, 2).reshape(T, DV + 1)
        p1 = np.concatenate([p1[TOWN:], p1[:TOWN]], axis=0)
        s = p0.astype(np.float64) + p1.astype(np.float64)
        outp[b] = (s[:, 0:DV] / s[:, DV : DV + 1] + bv).astype(np.float32)
    return outp


# revision 10
# speedup vs baseline: 1.0306x; 1.0306x over previous
"""Trainium2 Bass kernel for nn_AttentionLayer (4x2048x768, d_k=128, d_v=768).

Sharding (sequence-parallel over keys, data-parallel over batch):
8 cores; core c handles batch b=c//2 with KEY half h=c%2. Each core computes
q for ALL 2048 queries but k/v only for its own 1024 keys, then produces the
partial (unnormalized) attention numerator plus the partial softmax row sum:

    out_core[q, 0:768] = sum_{t in own half} exp(s_qt) * v[t, :]
    out_core[q, 768]   = sum_{t in own half} exp(s_qt)

The host adds the two partials of each batch and normalizes
(out = num/rowsum + bv) — an exact reassociation of the softmax.

bk is dropped entirely: s_it = (q̂_i+bq)·(k̂_t+bk) differs from
(q̂_i+bq)·k̂_t only by a per-query constant, which softmax cancels.

All matmul operands are bf16 (converted on the host); PE rate is the same
1 col/cycle as fp32r but all DMA bytes halve. Rel err vs fp32 ref ~2.6e-3.

DMA model (measured): each [128, X] DMA costs ~128 descriptors x ~16ns of
HW-DGE ring time regardless of X, so inputs are host-repacked partition-
major into the FEWEST, fattest-row DMAs, ordered by first PE use:
  scalar ring: wqk (3KB rows) | x-own c4|c5 pair (4KB) | wv whole (9KB)
  sync ring:   x-own c0|c1, c2|c3 pairs (4KB) | x-oth whole (12KB)
q/k consumes chunks in arrival order [0,1,4,5,2,3]. The output DRAM tensor
is partition-major [128, 16, 769] (host re-transposes) and region stores
alternate rings; the last tile's stores split by partition across rings.

PE: q/k-own -> [kT-cast(t); scores0(t); v(t)] interleaved (exp hides under
    v matmuls) -> q-other -> [scores1(t); out0(qc)] interleaved -> out1.
PSUM: "ps" pool 3x[128,1024] f32 (q/k/v/out, 6 banks) +
      "sc" pool 2x[128,512] f32 (score tiles, 2 banks) = all 8 banks.
"""

import sys

sys.path.insert(0, "/opt/trn_rl_repo")

import numpy as np
import ml_dtypes

B, T, DIN, DK, DV = 4, 2048, 768, 128, 768
NCORES = 8
TOWN = 1024  # own keys per core
CH = DIN // 128  # 6 contraction chunks over d_in
TCH = TOWN // 128  # 8 own-key chunks
QCH = T // 128  # 16 query chunks (all queries)
SCALE = 1.0 / float(np.sqrt(DK))

_CACHE = {}


def _build():
    from contextlib import ExitStack

    from concourse import bacc, mybir, tile

    f32 = mybir.dt.float32
    bf16 = mybir.dt.bfloat16

    nc = bacc.Bacc("TRN2", target_bir_lowering=False, debug=False)

    x_own = nc.dram_tensor("x_own", [128, 3, 2 * TOWN], bf16, kind="ExternalInput").ap()
    x_oth = nc.dram_tensor("x_oth", [128, CH, TOWN], bf16, kind="ExternalInput").ap()
    wqk = nc.dram_tensor("wqk", [128, CH, 2 * DK], bf16, kind="ExternalInput").ap()
    wv = nc.dram_tensor("wv", [128, CH, DV], bf16, kind="ExternalInput").ap()
    bq = nc.dram_tensor("bq", [DK, 1], f32, kind="ExternalInput").ap()
    out = nc.dram_tensor("out", [128, QCH, DV + 1], bf16, kind="ExternalOutput").ap()

    with tile.TileContext(nc) as tc, ExitStack() as ctx:
        consts = ctx.enter_context(tc.tile_pool(name="consts", bufs=1))
        persist = ctx.enter_context(tc.tile_pool(name="persist", bufs=1))
        wpool = ctx.enter_context(tc.tile_pool(name="wpool", bufs=1))
        xpool = ctx.enter_context(tc.tile_pool(name="xpool", bufs=1))
        out_pool = ctx.enter_context(tc.tile_pool(name="out_pool", bufs=4))
        ps_pool = ctx.enter_context(tc.tile_pool(name="ps", bufs=3, space="PSUM"))
        sc_pool = ctx.enter_context(tc.tile_pool(name="sc", bufs=2, space="PSUM"))

        bq_sb = consts.tile([DK, 1], f32)
        nc.gpsimd.dma_start(out=bq_sb[:], in_=bq)

        qT_sb = persist.tile([128, T], bf16)  # [dk, q] all queries, q̂+bq
        kT_sb = persist.tile([128, TOWN], bf16)  # [dk, t-own]
        v_sb = persist.tile([128, TCH, DV + 2], bf16)  # [t-part, chunk, dv|1|pad]
        pT_sb = persist.tile([128, TCH, T], bf16)  # [t-part, chunk, q]

        nc.vector.memset(v_sb[:, :, DV : DV + 2], 1.0)

        # x-own pairs: pair p holds chunks 2p|2p+1 side by side (4KB rows)
        xo_sb = xpool.tile([128, 3, 2 * TOWN], bf16)
        xt_sb = xpool.tile([128, CH, TOWN], bf16)
        wqk_sb = wpool.tile([128, CH, 2 * DK], bf16)
        wv_sb = wpool.tile([128, CH, DV], bf16)

        def xo(c):  # own-x chunk c -> [128, TOWN] slice of the pair tile
            return xo_sb[:, c // 2, (c % 2) * TOWN : (c % 2 + 1) * TOWN]

        # Input DMAs ordered by first PE use and balanced across both rings
        # (~0.2MB/us each): q/k chunks first, wv halves land just before the
        # v phase, x-other well before q-other
        nc.scalar.dma_start(out=wqk_sb[:], in_=wqk)
        nc.sync.dma_start(out=xo_sb[:, 0, :], in_=x_own[:, 0, :])
        nc.scalar.dma_start(out=xo_sb[:, 2, :], in_=x_own[:, 2, :])
        nc.sync.dma_start(out=xo_sb[:, 1, :], in_=x_own[:, 1, :])
        nc.sync.dma_start(out=wv_sb[:, 0:3, :], in_=wv[:, 0:3, :])
        nc.scalar.dma_start(out=wv_sb[:, 3:6, :], in_=wv[:, 3:6, :])
        nc.sync.dma_start(out=xt_sb[:, 0:3, :], in_=x_oth[:, 0:3, :])
        nc.scalar.dma_start(out=xt_sb[:, 3:6, :], in_=x_oth[:, 3:6, :])

        def emit_scores_t(t, qh):
            # scores^T for one own-key chunk, P^T = exp(scale*s) in bf16.
            # 512-col score tiles (1 PSUM bank each) so exps pipeline.
            for n0 in (0, 512):
                ps_s = sc_pool.tile([128, 512], f32, tag="sc")
                nc.tensor.matmul(
                    ps_s[:],
                    kT_sb[:, t * 128 : (t + 1) * 128],
                    qT_sb[:, qh * 1024 + n0 : qh * 1024 + n0 + 512],
                    start=True,
                    stop=True,
                )
                nc.scalar.activation(
                    pT_sb[:, t, qh * 1024 + n0 : qh * 1024 + n0 + 512],
                    ps_s[:],
                    mybir.ActivationFunctionType.Exp,
                    scale=SCALE,
                )

        def emit_v_t(t):
            # v-projection for one own-key chunk
            ps_v = ps_pool.tile([128, 1024], f32, tag="ps")
            for c in range(CH):
                for n0, n1 in ((0, 512), (512, DV)):
                    nc.tensor.matmul(
                        ps_v[:, n0:n1],
                        xo(c)[:, t * 128 : (t + 1) * 128],
                        wv_sb[:, c, n0:n1],
                        start=(c == 0),
                        stop=(c == CH - 1),
                    )
            nc.vector.tensor_copy(v_sb[:, t, 0:DV], ps_v[:, 0:DV])

        # qc pairs share one SBUF tile and one store DMA (contiguous 3KB
        # rows in the partition-major out tensor -> half the descriptor
        # load per ring); the last two tiles store solo/split so the drain
        # tail is short.
        o_state = {}

        def emit_out_qc(qc):
            # partial numerator + rowsum: out[qc] = sum_t P^T[t,qc].T @ [v|1]
            ps_o = ps_pool.tile([128, 1024], f32, tag="ps")
            if qc % 2 == 0:
                o_pair = out_pool.tile([128, 2, DV + 1], bf16, tag="o")
                o_state["tile"] = o_pair
            o_sb = o_state["tile"][:, qc % 2, :]
            for reg, (n0, n1) in enumerate(((0, 512), (512, DV + 2))):
                for t in range(TCH):
                    nc.tensor.matmul(
                        ps_o[:, n0:n1],
                        pT_sb[:, t, qc * 128 : (qc + 1) * 128],
                        v_sb[:, t, n0:n1],
                        start=(t == 0),
                        stop=(t == TCH - 1),
                    )
                c1 = min(n1, DV + 1)
                nc.vector.tensor_copy(o_sb[:, n0:c1], ps_o[:, n0:c1])
                if qc == QCH - 2 and reg == 1:
                    # penultimate tile: store alone so only the last tile's
                    # bytes remain after the final matmul
                    nc.sync.dma_start(
                        out=out[:, qc, :], in_=o_sb[:, 0 : DV + 1]
                    )
                elif qc == QCH - 1:
                    if reg == 0:
                        nc.scalar.dma_start(
                            out=out[:, qc, n0:c1], in_=o_sb[:, n0:c1]
                        )
                    else:
                        # final region: split by partition across both rings
                        nc.sync.dma_start(
                            out=out[0:64, qc, n0:c1], in_=o_sb[0:64, n0:c1]
                        )
                        nc.scalar.dma_start(
                            out=out[64:128, qc, n0:c1], in_=o_sb[64:128, n0:c1]
                        )
                elif qc % 2 == 1 and reg == 1:
                    # pair complete: one contiguous 2-tile store
                    eng = nc.sync if (qc // 2) % 2 == 0 else nc.scalar
                    eng.dma_start(
                        out=out[:, qc - 1 : qc + 1, :], in_=o_state["tile"][:]
                    )

        # q own-half then k own, each a single run of region-alternating mms
        # into ONE psum tile: switching psum tiles costs the PE a ~280ns
        # pipeline flush, so q and k are serial blocks (1 switch) instead of
        # interleaved (12 switches). Chunk order matches DMA arrival order.
        ps_q0 = ps_pool.tile([128, 1024], f32, tag="ps")
        ps_k = ps_pool.tile([128, 1024], f32, tag="ps")
        C_ORDER = [0, 1, 4, 5, 2, 3]
        for dst, w0 in ((ps_q0, 0), (ps_k, DK)):
            for i, c in enumerate(C_ORDER):
                for n0 in (0, 512):
                    nc.tensor.matmul(
                        dst[:, n0 : n0 + 512],
                        wqk_sb[:, c, w0 : w0 + DK],
                        xo(c)[:, n0 : n0 + 512],
                        start=(i == 0),
                        stop=(i == CH - 1),
                    )
        # qT = q̂+bq on scalar, split per 512 so scores t=0 unblocks early;
        # kT casts are per-chunk inside the loop below for the same reason
        for n0 in (0, 512):
            nc.scalar.activation(
                qT_sb[:, n0 : n0 + 512],
                ps_q0[:, n0 : n0 + 512],
                mybir.ActivationFunctionType.Identity,
                bias=bq_sb[:],
            )

        # scores for own queries interleaved with v-projection: the scalar
        # exps (~1.4us/chunk) hide under the v matmuls (~2.4us/chunk)
        for t in range(TCH):
            nc.vector.tensor_copy(
                kT_sb[:, t * 128 : (t + 1) * 128], ps_k[:, t * 128 : (t + 1) * 128]
            )
            emit_scores_t(t, 0)
            emit_v_t(t)

        # q other-half
        ps_q1 = ps_pool.tile([128, 1024], f32, tag="ps")
        for c in range(CH):
            for n0 in (0, 512):
                nc.tensor.matmul(
                    ps_q1[:, n0 : n0 + 512],
                    wqk_sb[:, c, 0:DK],
                    xt_sb[:, c, n0 : n0 + 512],
                    start=(c == 0),
                    stop=(c == CH - 1),
                )
        for n0 in (0, 512):
            nc.scalar.activation(
                qT_sb[:, TOWN + n0 : TOWN + n0 + 512],
                ps_q1[:, n0 : n0 + 512],
                mybir.ActivationFunctionType.Identity,
                bias=bq_sb[:],
            )

        # scores for other-half queries interleaved with out0: exps for
        # half 1 complete long before out1 consumes pT, with no PE stall
        for qc in range(8):
            emit_scores_t(qc, 1)
            emit_out_qc(qc)

        for qc in range(8, 16):
            emit_out_qc(qc)

    nc.compile()
    return nc


def _get_nc():
    if "nc" not in _CACHE:
        _CACHE["nc"] = _build()
    return _CACHE["nc"]


def _make_in_maps(x, Wq, bq, Wk, bk, Wv):
    bf16 = ml_dtypes.bfloat16
    wq = np.asarray(Wq, np.float32).astype(bf16).reshape(CH, 128, DK)
    wk = np.asarray(Wk, np.float32).astype(bf16).reshape(CH, 128, DK)
    base = {
        # partition-major packs: contiguous per-partition rows -> few, fat
        # DMA descriptors (wqk 3KB, wv 9KB, x-own 4KB, x-oth 12KB rows)
        "wqk": np.ascontiguousarray(
            np.concatenate([wq, wk], axis=2).transpose(1, 0, 2)
        ),
        "wv": np.ascontiguousarray(
            np.asarray(Wv, np.float32).astype(bf16).reshape(CH, 128, DV).transpose(1, 0, 2)
        ),
        "bq": np.ascontiguousarray(np.asarray(bq, np.float32).reshape(DK, 1)),
    }
    in_maps = []
    for c in range(NCORES):
        b, h = c // 2, c % 2
        xb = x[b]  # [T, DIN]
        rot = np.concatenate([xb[h * TOWN :], xb[: h * TOWN]], axis=0)
        xT = rot.T.astype(bf16).reshape(CH, 128, T).transpose(1, 0, 2)  # [128,c,t]
        own = xT[:, :, 0:TOWN]  # [128, c, 1024]
        m = dict(base)
        m["x_own"] = np.ascontiguousarray(own.reshape(128, 3, 2 * TOWN))
        m["x_oth"] = np.ascontiguousarray(xT[:, :, TOWN:T])
        in_maps.append(m)
    return in_maps


def kernel(x, Wq, bq, Wk, bk, Wv, bv):
    from concourse import bass_utils

    x = np.ascontiguousarray(np.asarray(x, dtype=np.float32))
    nc = _get_nc()
    in_maps = _make_in_maps(x, Wq, bq, Wk, bk, Wv)

    res = bass_utils.run_bass_kernel_spmd(nc, in_maps, core_ids=list(range(NCORES)))

    bv = np.asarray(bv, np.float32).reshape(1, DV)
    outp = np.empty((B, T, DV), dtype=np.float32)
    for b in range(B):
        # out is partition-major [128, qc, 769] -> [qc*128+p, 769]
        p0 = res.results[2 * b]["out"].transpose(1, 0, 2).reshape(T, DV + 1)
        p1 = res.results[2 * b + 1]["out"].transpose(1, 0, 2).reshape(T, DV + 1)
        p1 = np.concatenate([p1[TOWN:], p1[:TOWN]], axis=0)
        s = p0.astype(np.float64) + p1.astype(np.float64)
        outp[b] = (s[:, 0:DV] / s[:, DV : DV + 1] + bv).astype(np.float32)
    return outp


# revision 14
# speedup vs baseline: 1.0526x; 1.0213x over previous
"""Trainium2 Bass kernel for nn_AttentionLayer (4x2048x768, d_k=128, d_v=768).

Sharding (sequence-parallel over keys, data-parallel over batch):
8 cores; core c handles batch b=c//2 with KEY half h=c%2. Each core computes
q for ALL 2048 queries but k/v only for its own 1024 keys, then produces the
partial (unnormalized) attention numerator plus the partial softmax row sum:

    out_core[q, 0:768] = sum_{t in own half} exp(s_qt) * v[t, :]
    out_core[q, 768]   = sum_{t in own half} exp(s_qt)

The host adds the two partials of each batch and normalizes
(out = num/rowsum + bv) — an exact reassociation of the softmax.

bk is dropped entirely: s_it = (q̂_i+bq)·(k̂_t+bk) differs from
(q̂_i+bq)·k̂_t only by a per-query constant, which softmax cancels.

All matmul operands are bf16 (converted on the host); PE rate is the same
1 col/cycle as fp32r but all DMA bytes halve. Rel err vs fp32 ref ~2.6e-3.

DMA model (measured): each [128, X] DMA costs ~128 descriptors x ~16ns of
HW-DGE ring time regardless of X, so inputs are host-repacked partition-
major into the FEWEST, fattest-row DMAs, ordered by first PE use:
  scalar ring: wqk (3KB rows) | x-own c4|c5 pair (4KB) | wv whole (9KB)
  sync ring:   x-own c0|c1, c2|c3 pairs (4KB) | x-oth whole (12KB)
q/k consumes chunks in arrival order [0,1,4,5,2,3]. The output DRAM tensor
is partition-major [128, 16, 769] (host re-transposes) and region stores
alternate rings; the last tile's stores split by partition across rings.

PE: q/k-own -> [kT-cast(t); scores0(t); v(t)] interleaved (exp hides under
    v matmuls) -> q-other -> [scores1(t); out0(qc)] interleaved -> out1.
PSUM: "ps" pool 3x[128,1024] f32 (q/k/v/out, 6 banks) +
      "sc" pool 2x[128,512] f32 (score tiles, 2 banks) = all 8 banks.
"""

import sys

sys.path.insert(0, "/opt/trn_rl_repo")

import numpy as np
import ml_dtypes

B, T, DIN, DK, DV = 4, 2048, 768, 128, 768
NCORES = 8
TOWN = 1024  # own keys per core
CH = DIN // 128  # 6 contraction chunks over d_in
TCH = TOWN // 128  # 8 own-key chunks
QCH = T // 128  # 16 query chunks (all queries)
SCALE = 1.0 / float(np.sqrt(DK))

_CACHE = {}


def _build():
    from contextlib import ExitStack

    from concourse import bacc, mybir, tile

    f32 = mybir.dt.float32
    bf16 = mybir.dt.bfloat16

    nc = bacc.Bacc("TRN2", target_bir_lowering=False, debug=False)

    x_own = nc.dram_tensor("x_own", [128, 3, 2 * TOWN], bf16, kind="ExternalInput").ap()
    x_oth = nc.dram_tensor("x_oth", [128, CH, TOWN], bf16, kind="ExternalInput").ap()
    wqk = nc.dram_tensor("wqk", [128, CH, 2 * DK], bf16, kind="ExternalInput").ap()
    wv = nc.dram_tensor("wv", [128, CH, DV], bf16, kind="ExternalInput").ap()
    bq = nc.dram_tensor("bq", [DK, 1], f32, kind="ExternalInput").ap()
    out = nc.dram_tensor("out", [128, QCH, DV + 1], bf16, kind="ExternalOutput").ap()

    with tile.TileContext(nc) as tc, ExitStack() as ctx:
        consts = ctx.enter_context(tc.tile_pool(name="consts", bufs=1))
        persist = ctx.enter_context(tc.tile_pool(name="persist", bufs=1))
        wpool = ctx.enter_context(tc.tile_pool(name="wpool", bufs=1))
        xpool = ctx.enter_context(tc.tile_pool(name="xpool", bufs=1))
        out_pool = ctx.enter_context(tc.tile_pool(name="out_pool", bufs=4))
        ps_pool = ctx.enter_context(tc.tile_pool(name="ps", bufs=3, space="PSUM"))
        sc_pool = ctx.enter_context(tc.tile_pool(name="sc", bufs=2, space="PSUM"))

        bq_sb = consts.tile([DK, 1], f32)
        nc.gpsimd.dma_start(out=bq_sb[:], in_=bq)

        qT_sb = persist.tile([128, T], bf16)  # [dk, q] all queries, q̂+bq
        kT_sb = persist.tile([128, TOWN], bf16)  # [dk, t-own]
        v_sb = persist.tile([128, TCH, DV + 2], bf16)  # [t-part, chunk, dv|1|pad]
        pT_sb = persist.tile([128, TCH, T], bf16)  # [t-part, chunk, q]

        nc.vector.memset(v_sb[:, :, DV : DV + 2], 1.0)

        # x-own pairs: pair p holds chunks 2p|2p+1 side by side (4KB rows)
        xo_sb = xpool.tile([128, 3, 2 * TOWN], bf16)
        xt_sb = xpool.tile([128, CH, TOWN], bf16)
        wqk_sb = wpool.tile([128, CH, 2 * DK], bf16)
        wv_sb = wpool.tile([128, CH, DV], bf16)

        def xo(c):  # own-x chunk c -> [128, TOWN] slice of the pair tile
            return xo_sb[:, c // 2, (c % 2) * TOWN : (c % 2 + 1) * TOWN]

        # PE p-state warm-up: the tensor engine runs its first ~3us at a
        # reduced clock after idling; burn that ramp on throwaway matmuls
        # during the unavoidable first-DMA wait so the real q/k phase runs
        # at full speed. (Emitted first so the memset leads the vector
        # queue.)
        warm = consts.tile([128, 640], bf16)
        nc.vector.memset(warm[:], 0.0)
        for _ in range(16):
            ps_w = sc_pool.tile([128, 512], f32, tag="sc")
            nc.tensor.matmul(
                ps_w[:], warm[:, 0:128], warm[:, 128:640], start=True, stop=True
            )

        # Input DMAs: a ring shares its ~0.17MB/us bandwidth over ALL queued
        # DMAs, so only the q/k-critical loads are issued up front; wv and
        # x-oth are held back behind an artificial WAW dep (a 1-element copy
        # whose source is the first x pair) so they don't steal bandwidth
        # from the chunks the q/k phase is waiting on.
        nc.scalar.dma_start(out=wqk_sb[:], in_=wqk)
        nc.sync.dma_start(out=xo_sb[:, 0, :], in_=x_own[:, 0, :])
        nc.scalar.dma_start(out=xo_sb[:, 2, :], in_=x_own[:, 2, :])
        nc.sync.dma_start(out=xo_sb[:, 1, :], in_=x_own[:, 1, :])
        nc.vector.tensor_copy(wv_sb[:, 0, 0:1], xo_sb[:, 0, 0:1])
        nc.vector.tensor_copy(wv_sb[:, 3, 0:1], xo_sb[:, 0, 0:1])
        nc.sync.dma_start(out=wv_sb[:, 0:3, :], in_=wv[:, 0:3, :])
        nc.scalar.dma_start(out=wv_sb[:, 3:6, :], in_=wv[:, 3:6, :])
        nc.vector.tensor_copy(xt_sb[:, 0, 0:1], xo_sb[:, 1, 0:1])
        nc.vector.tensor_copy(xt_sb[:, 3, 0:1], xo_sb[:, 1, 0:1])
        nc.sync.dma_start(out=xt_sb[:, 0:3, :], in_=x_oth[:, 0:3, :])
        nc.scalar.dma_start(out=xt_sb[:, 3:6, :], in_=x_oth[:, 3:6, :])

        def emit_scores_t(t, qh):
            # scores^T for one own-key chunk, P^T = exp(scale*s) in bf16.
            # 512-col score tiles (1 PSUM bank each) so exps pipeline.
            for n0 in (0, 512):
                ps_s = sc_pool.tile([128, 512], f32, tag="sc")
                nc.tensor.matmul(
                    ps_s[:],
                    kT_sb[:, t * 128 : (t + 1) * 128],
                    qT_sb[:, qh * 1024 + n0 : qh * 1024 + n0 + 512],
                    start=True,
                    stop=True,
                )
                nc.scalar.activation(
                    pT_sb[:, t, qh * 1024 + n0 : qh * 1024 + n0 + 512],
                    ps_s[:],
                    mybir.ActivationFunctionType.Exp,
                    scale=SCALE,
                )

        def emit_v_t(t):
            # v-projection for one own-key chunk
            ps_v = ps_pool.tile([128, 1024], f32, tag="ps")
            for c in range(CH):
                for n0, n1 in ((0, 512), (512, DV)):
                    nc.tensor.matmul(
                        ps_v[:, n0:n1],
                        xo(c)[:, t * 128 : (t + 1) * 128],
                        wv_sb[:, c, n0:n1],
                        start=(c == 0),
                        stop=(c == CH - 1),
                    )
            nc.vector.tensor_copy(v_sb[:, t, 0:DV], ps_v[:, 0:DV])

        # qc pairs share one SBUF tile and one store DMA (contiguous 3KB
        # rows in the partition-major out tensor -> half the descriptor
        # load per ring); the last two tiles store solo/split so the drain
        # tail is short.
        o_state = {}

        def emit_out_qc(qc):
            # partial numerator + rowsum: out[qc] = sum_t P^T[t,qc].T @ [v|1]
            ps_o = ps_pool.tile([128, 1024], f32, tag="ps")
            if qc % 2 == 0:
                o_pair = out_pool.tile([128, 2, DV + 1], bf16, tag="o")
                o_state["tile"] = o_pair
            o_sb = o_state["tile"][:, qc % 2, :]
            for reg, (n0, n1) in enumerate(((0, 512), (512, DV + 2))):
                for t in range(TCH):
                    nc.tensor.matmul(
                        ps_o[:, n0:n1],
                        pT_sb[:, t, qc * 128 : (qc + 1) * 128],
                        v_sb[:, t, n0:n1],
                        start=(t == 0),
                        stop=(t == TCH - 1),
                    )
                c1 = min(n1, DV + 1)
                nc.vector.tensor_copy(o_sb[:, n0:c1], ps_o[:, n0:c1])
                if qc >= QCH - 2:
                    # last two tiles: store each region immediately, split
                    # by partition across both rings, so the post-matmul
                    # drain carries as few bytes as possible per ring
                    nc.sync.dma_start(
                        out=out[0:64, qc, n0:c1], in_=o_sb[0:64, n0:c1]
                    )
                    nc.scalar.dma_start(
                        out=out[64:128, qc, n0:c1], in_=o_sb[64:128, n0:c1]
                    )
                elif qc % 2 == 1 and reg == 1:
                    # pair complete: one contiguous 2-tile store
                    eng = nc.sync if (qc // 2) % 2 == 0 else nc.scalar
                    eng.dma_start(
                        out=out[:, qc - 1 : qc + 1, :], in_=o_state["tile"][:]
                    )

        # q own-half then k own, each a single run of region-alternating mms
        # into ONE psum tile: switching psum tiles costs the PE a ~280ns
        # pipeline flush, so q and k are serial blocks (1 switch) instead of
        # interleaved (12 switches). Chunk order matches DMA arrival order.
        ps_q0 = ps_pool.tile([128, 1024], f32, tag="ps")
        ps_k = ps_pool.tile([128, 1024], f32, tag="ps")
        C_ORDER = [0, 1, 4, 5, 2, 3]
        for dst, w0 in ((ps_q0, 0), (ps_k, DK)):
            for i, c in enumerate(C_ORDER):
                for n0 in (0, 512):
                    nc.tensor.matmul(
                        dst[:, n0 : n0 + 512],
                        wqk_sb[:, c, w0 : w0 + DK],
                        xo(c)[:, n0 : n0 + 512],
                        start=(i == 0),
                        stop=(i == CH - 1),
                    )
        # qT = q̂+bq on scalar, split per 512 so scores t=0 unblocks early;
        # kT casts are per-chunk inside the loop below for the same reason
        for n0 in (0, 512):
            nc.scalar.activation(
                qT_sb[:, n0 : n0 + 512],
                ps_q0[:, n0 : n0 + 512],
                mybir.ActivationFunctionType.Identity,
                bias=bq_sb[:],
            )

        # scores for own queries interleaved with v-projection: the scalar
        # exps (~1.4us/chunk) hide under the v matmuls (~2.4us/chunk)
        for t in range(TCH):
            nc.vector.tensor_copy(
                kT_sb[:, t * 128 : (t + 1) * 128], ps_k[:, t * 128 : (t + 1) * 128]
            )
            emit_scores_t(t, 0)
            emit_v_t(t)

        # q other-half
        ps_q1 = ps_pool.tile([128, 1024], f32, tag="ps")
        for c in range(CH):
            for n0 in (0, 512):
                nc.tensor.matmul(
                    ps_q1[:, n0 : n0 + 512],
                    wqk_sb[:, c, 0:DK],
                    xt_sb[:, c, n0 : n0 + 512],
                    start=(c == 0),
                    stop=(c == CH - 1),
                )
        for n0 in (0, 512):
            nc.scalar.activation(
                qT_sb[:, TOWN + n0 : TOWN + n0 + 512],
                ps_q1[:, n0 : n0 + 512],
                mybir.ActivationFunctionType.Identity,
                bias=bq_sb[:],
            )

        # scores for other-half queries interleaved with out0: exps for
        # half 1 complete long before out1 consumes pT, with no PE stall
        for qc in range(8):
            emit_scores_t(qc, 1)
            emit_out_qc(qc)

        for qc in range(8, 16):
            emit_out_qc(qc)

    nc.compile()
    return nc


def _get_nc():
    if "nc" not in _CACHE:
        _CACHE["nc"] = _build()
    return _CACHE["nc"]


def _make_in_maps(x, Wq, bq, Wk, bk, Wv):
    bf16 = ml_dtypes.bfloat16
    wq = np.asarray(Wq, np.float32).astype(bf16).reshape(CH, 128, DK)
    wk = np.asarray(Wk, np.float32).astype(bf16).reshape(CH, 128, DK)
    base = {
        # partition-major packs: contiguous per-partition rows -> few, fat
        # DMA descriptors (wqk 3KB, wv 9KB, x-own 4KB, x-oth 12KB rows)
        "wqk": np.ascontiguousarray(
            np.concatenate([wq, wk], axis=2).transpose(1, 0, 2)
        ),
        "wv": np.ascontiguousarray(
            np.asarray(Wv, np.float32).astype(bf16).reshape(CH, 128, DV).transpose(1, 0, 2)
        ),
        "bq": np.ascontiguousarray(np.asarray(bq, np.float32).reshape(DK, 1)),
    }
    in_maps = []
    for c in range(NCORES):
        b, h = c // 2, c % 2
        xb = x[b]  # [T, DIN]
        rot = np.concatenate([xb[h * TOWN :], xb[: h * TOWN]], axis=0)
        xT = rot.T.astype(bf16).reshape(CH, 128, T).transpose(1, 0, 2)  # [128,c,t]
        own = xT[:, :, 0:TOWN]  # [128, c, 1024]
        m = dict(base)
        m["x_own"] = np.ascontiguousarray(own.reshape(128, 3, 2 * TOWN))
        m["x_oth"] = np.ascontiguousarray(xT[:, :, TOWN:T])
        in_maps.append(m)
    return in_maps


def kernel(x, Wq, bq, Wk, bk, Wv, bv):
    from concourse import bass_utils

    x = np.ascontiguousarray(np.asarray(x, dtype=np.float32))
    nc = _get_nc()
    in_maps = _make_in_maps(x, Wq, bq, Wk, bk, Wv)

    res = bass_utils.run_bass_kernel_spmd(nc, in_maps, core_ids=list(range(NCORES)))

    bv = np.asarray(bv, np.float32).reshape(1, DV)
    outp = np.empty((B, T, DV), dtype=np.float32)
    for b in range(B):
        # out is partition-major [128, qc, 769] -> [qc*128+p, 769]
        p0 = res.results[2 * b]["out"].transpose(1, 0, 2).reshape(T, DV + 1)
        p1 = res.results[2 * b + 1]["out"].transpose(1, 0, 2).reshape(T, DV + 1)
        p1 = np.concatenate([p1[TOWN:], p1[:TOWN]], axis=0)
        s = p0.astype(np.float64) + p1.astype(np.float64)
        outp[b] = (s[:, 0:DV] / s[:, DV : DV + 1] + bv).astype(np.float32)
    return outp
